# revision 22
# baseline (speedup 1.0000x reference)
"""Multi-head causal attention (B=4, S=2048, D=1024, H=16) on 8 trn2 NeuronCores.

Sharding: data-parallel over batch (4) x tensor-parallel over heads (2 groups
of 8).  Core c = (b, g) computes, for batch b, head group g:
  QT = (Wq_g^T X_b^T + bq_g)           [512, 2048]  (bf16, head_dim on partitions)
  KT likewise; V = X_b Wv_g + bv_g     [2048, 512]  (bf16, ones column per head)
  per head h, query tile qt (128):
     S^T[k, q] = Kh^T^T Qh^T           (bf16 matmuls, causal block-skipped)
     A^T = exp(S^T / 8)                (ScalarE, k-tiles packed along psum free dim)
     diagonal block masked via bf16 multiply
     psum_o[q, 65] += A^T.T @ V'_h     (col 64 = row sums via ones column)
     O[q, h*64:+64] = psum_o[:, :64] * recip(psum_o[:, 64])
  O^T via PE transpose, out^T = Wo_g^T O^T  (partial, fp32)
Host sums the two per-batch partials and adds bo.
"""

import math

import numpy as np

B, S, D, H = 4, 2048, 1024, 16
HD = D // H          # 64
NCORES = 8
HPC = 8              # heads per core
DM = HPC * HD        # 512 mid-dims per core
NQT = S // 128       # 16 query tiles
KT_PER_EXP = 8       # k-tiles packed into one [128, 1024] psum before exp
VROW = HD + 1        # 65: per-head V columns incl. ones column

_CACHE = {}
DEBUG = False


def _build_program():
    import concourse.mybir as mybir
    import concourse.tile as tile
    from concourse import bacc

    f32 = mybir.dt.float32
    f32r = mybir.dt.float32r
    bf16 = mybir.dt.bfloat16
    EXP = mybir.ActivationFunctionType.Exp

    nc = bacc.Bacc("TRN2", target_bir_lowering=False, debug=False,
                   num_devices=NCORES)

    xqT_d = nc.dram_tensor("xqT", [D, S], f32r, kind="ExternalInput")
    xkT_d = nc.dram_tensor("xkT", [D, S], f32r, kind="ExternalInput")
    xvT_d = nc.dram_tensor("xvT", [D, S], f32r, kind="ExternalInput")
    wq_d = nc.dram_tensor("wq", [D, DM], f32r, kind="ExternalInput")
    wk_d = nc.dram_tensor("wk", [D, DM], f32r, kind="ExternalInput")
    wv_d = nc.dram_tensor("wv", [D, DM], f32r, kind="ExternalInput")
    bq_d = nc.dram_tensor("bq", [128, 4], f32, kind="ExternalInput")
    bk_d = nc.dram_tensor("bk", [128, 4], f32, kind="ExternalInput")
    bvb_d = nc.dram_tensor("bvb", [128, DM], f32, kind="ExternalInput")
    wo_d = nc.dram_tensor("wo", [DM, D], bf16, kind="ExternalInput")
    cmask_d = nc.dram_tensor("cmask", [128, 256], bf16, kind="ExternalInput")
    ident_d = nc.dram_tensor("ident", [128, 128], bf16, kind="ExternalInput")
    outT_d = nc.dram_tensor("outT", [D, S], f32, kind="ExternalOutput")

    with tile.TileContext(nc) as tc:
        with (
            tc.tile_pool(name="res", bufs=1) as res,     # long-lived tensors
            tc.tile_pool(name="wrk", bufs=1) as wrk,     # rotating work tiles
            tc.tile_pool(name="ps", bufs=1, space="PSUM") as ps,
        ):
            # ---- resident tensors -------------------------------------
            wq_sb = res.tile([128, 8 * DM], f32r, tag="wq_sb")
            wk_sb = res.tile([128, 8 * DM], f32r, tag="wk_sb")
            wv_sb = res.tile([128, 8 * DM], f32r, tag="wv_sb")
            wo_sb = res.tile([128, 4 * D], bf16, tag="wo_sb")
            bq_sb = res.tile([128, 4], f32, tag="bq_sb")
            bk_sb = res.tile([128, 4], f32, tag="bk_sb")
            bvb_sb = res.tile([128, DM], f32, tag="bvb_sb")
            cmask_sb = res.tile([128, 256], bf16, tag="cmask_sb")
            ident_sb = res.tile([128, 128], bf16, tag="ident_sb")
            qT_sb = [res.tile([128, S], bf16, tag=f"qT{m}", name=f"qT{m}") for m in range(4)]
            kT_sb = [res.tile([128, S], bf16, tag=f"kT{m}", name=f"kT{m}") for m in range(4)]
            v_sb = res.tile([128, NQT * HPC * VROW], bf16, tag="v_sb")
            oT_sb = [res.tile([128, S], bf16, tag=f"oT{m}", name=f"oT{m}") for m in range(4)]

            def load_w(w_sb, w_d, n_w):
                nc.sync.dma_start(
                    w_sb.rearrange("p (k n) -> p k n", n=n_w),
                    w_d.rearrange("(k p) n -> p k n", p=128),
                )

            v4 = v_sb.rearrange("p (s h c) -> p s h c", h=HPC, c=VROW)
            bvb3 = bvb_sb.rearrange("p (h c) -> p h c", h=HPC)

            # ---- projections, split into per-m-tile units -------------
            def load_xch(xT_d, n):
                xch = wrk.tile([128, 8 * 512], f32r, tag="xch", name="xch",
                               bufs=2)
                nc.sync.dma_start(
                    xch.rearrange("p (k s) -> p k s", k=8),
                    xT_d.rearrange("(k p) s -> p k s", p=128)[
                        :, :, n * 512 : (n + 1) * 512
                    ],
                )
                return xch

            def proj_qk_unit(xch, w_sb, b_sb, dst, n, m):
                pp = ps.tile([128, 512], f32, tag="ps_small", name="pp",
                             bufs=2)
                for kt in range(8):
                    nc.tensor.matmul(
                        pp[:],
                        w_sb[:, kt * DM + m * 128 : kt * DM + (m + 1) * 128],
                        xch[:, kt * 512 : (kt + 1) * 512],
                        start=(kt == 0),
                        stop=(kt == 7),
                    )
                nc.vector.tensor_scalar_add(
                    dst[m][:, n * 512 : (n + 1) * 512], pp[:],
                    b_sb[:, m : m + 1],
                )

            def proj_v_unit(xch, n, mi):
                st = n * 4 + mi          # S tile index
                pp = ps.tile([128, 512], f32, tag="ps_small", name="pp",
                             bufs=2)
                for kt in range(8):
                    nc.tensor.matmul(
                        pp[:],
                        xch[:, kt * 512 + mi * 128 : kt * 512 + (mi + 1) * 128],
                        wv_sb[:, kt * DM : (kt + 1) * DM],
                        start=(kt == 0),
                        stop=(kt == 7),
                    )
                # add bias + scatter heads into 65-wide groups
                nc.vector.tensor_add(
                    v4[:, st, :, 0:HD],
                    pp.rearrange("p (h c) -> p h c", h=HPC),
                    bvb3[:],
                )

            def proj_units(n):
                xq = load_xch(xqT_d, n)
                for m in range(4):
                    yield lambda m=m, x=xq: proj_qk_unit(x, wq_sb, bq_sb,
                                                         qT_sb, n, m)
                xk = load_xch(xkT_d, n)
                for m in range(4):
                    yield lambda m=m, x=xk: proj_qk_unit(x, wk_sb, bk_sb,
                                                         kT_sb, n, m)
                xv = load_xch(xvT_d, n)
                for mi in range(4):
                    yield lambda mi=mi, x=xv: proj_v_unit(x, n, mi)

            # ---- interleaved phases: per S-chunk n: projections,
            # ---- attention on its 4 query tiles, output projection ----
            def attention(hp, qt, o_nat):
                nblk = qt + 1
                heads = (2 * hp, 2 * hp + 1)
                aT = [
                    wrk.tile([128, S], bf16, tag=f"aT{hh}", name=f"aT{hh}", bufs=3)
                    for hh in range(2)
                ]
                # both heads in one 2-bank psum (separate banks: matmul
                # start=True clears the whole bank's has_written flags)
                po = ps.tile([128, 1024], f32, tag="ps_o", name="po", bufs=1)
                pof = [po[:, 0:VROW], po[:, 512 : 512 + VROW]]

                def mm1_exp(blk, cnt):
                    # the two heads sit on PE row-groups 0-63 / 64-127;
                    # alternating their matmuls lets the sub-arrays run
                    # them concurrently (per-subarray LDWEIGHTS overlap)
                    psx = [
                        ps.tile([128, 1024], f32, tag="ps_s",
                                name=f"psx{hh}", bufs=2)
                        for hh in range(2)
                    ]
                    for j in range(cnt):
                        kt = blk * KT_PER_EXP + j
                        for hh in range(2):
                            p0 = hh * 64
                            nc.tensor.matmul(
                                psx[hh][:, j * 128 : (j + 1) * 128],
                                kT_sb[hp][p0 : p0 + 64, kt * 128 : (kt + 1) * 128],
                                qT_sb[hp][p0 : p0 + 64, qt * 128 : (qt + 1) * 128],
                                start=True,
                                stop=True,
                            )
                    for hh in range(2):
                        nc.scalar.activation(
                            aT[hh][:, blk * 1024 : blk * 1024 + cnt * 128],
                            psx[hh][:, : cnt * 128],
                            EXP,
                            scale=1.0 / math.sqrt(HD),
                        )

                def mm2(blk, cnt):
                    for hh in range(2):
                        for j in range(cnt):
                            kt = blk * KT_PER_EXP + j
                            nc.tensor.matmul(
                                pof[hh],
                                aT[hh][:, kt * 128 : (kt + 1) * 128],
                                v4[:, kt, heads[hh], :],
                                start=(kt == 0),
                                stop=(kt == nblk - 1),
                                skip_group_check=True,
                            )

                nexp = (nblk + KT_PER_EXP - 1) // KT_PER_EXP
                cnts = [min(KT_PER_EXP, nblk - b * KT_PER_EXP)
                        for b in range(nexp)]
                for blk in range(nexp):
                    mm1_exp(blk, cnts[blk])
                    if blk == nexp - 1:
                        for hh in range(2):
                            dslc = aT[hh][:, qt * 128 : (qt + 1) * 128]
                            nc.vector.tensor_mul(dslc, dslc,
                                                 cmask_sb[:, 0:128])
                    if blk > 0:
                        mm2(blk - 1, cnts[blk - 1])
                mm2(nexp - 1, cnts[nexp - 1])

                rc = wrk.tile([128, 2], f32, tag="rc", name="rc", bufs=2)
                po_sums = po.rearrange("p (b c) -> p b c", c=512)[:, :, HD : HD + 1]
                nc.vector.reciprocal(rc[:], po_sums)
                for hh in range(2):
                    nc.vector.tensor_scalar_mul(
                        o_nat[:, heads[hh] * HD : (heads[hh] + 1) * HD],
                        pof[hh][:, 0:HD],
                        rc[:, hh : hh + 1],
                    )
                if DEBUG and hp == 0 and qt == 15:
                    for hh in range(2):
                        da = nc.dram_tensor(f"dbg_aT{hh}", [128, S], bf16,
                                            kind="ExternalOutput")
                        nc.sync.dma_start(da[:], aT[hh][:])
                        dpo = nc.dram_tensor(f"dbg_po{hh}", [128, VROW], f32,
                                             kind="ExternalOutput")
                        pos = wrk.tile([128, VROW], f32, tag="dbgpo",
                                       name="dbgpo", bufs=2)
                        nc.vector.tensor_copy(pos[:], pof[hh])
                        nc.sync.dma_start(dpo[:], pos[:])

            def transpose_o(qt, o_nat):
                for m in range(4):
                    pt = ps.tile([128, 512], bf16, tag="ps_small", bufs=2)
                    nc.tensor.transpose(
                        pt[:, :128],
                        o_nat[:, m * 128 : (m + 1) * 128],
                        ident_sb[:],
                    )
                    nc.vector.tensor_copy(
                        oT_sb[m][:, qt * 128 : (qt + 1) * 128], pt[:, :128]
                    )

            def outproj_unit(n, m8):
                pp = ps.tile([128, 512], f32, tag="ps_small", name="pp",
                             bufs=2)
                for kt in range(4):
                    nc.tensor.matmul(
                        pp[:],
                        wo_sb[:, kt * D + m8 * 128 : kt * D + (m8 + 1) * 128],
                        oT_sb[kt][:, n * 512 : (n + 1) * 512],
                        start=(kt == 0),
                        stop=(kt == 3),
                    )
                ost = wrk.tile([128, 512], f32, tag="ost", name="ost", bufs=2)
                nc.vector.tensor_copy(ost[:], pp[:])
                nc.sync.dma_start(
                    outT_d[m8 * 128 : (m8 + 1) * 128, n * 512 : (n + 1) * 512],
                    ost[:],
                )

            def outproj_units(n):
                for m8 in range(8):
                    yield lambda m8=m8: outproj_unit(n, m8)

            # batch 0: emit projections for n=0 up front, DMAs ordered
            # so the first matmul's operands arrive first
            units0 = []
            load_w(wq_sb, wq_d, DM)
            nc.sync.dma_start(bq_sb[:], bq_d[:])
            xq = load_xch(xqT_d, 0)
            for m in range(4):
                proj_qk_unit(xq, wq_sb, bq_sb, qT_sb, 0, m)
            load_w(wk_sb, wk_d, DM)
            nc.sync.dma_start(bk_sb[:], bk_d[:])
            xk = load_xch(xkT_d, 0)
            for m in range(4):
                proj_qk_unit(xk, wk_sb, bk_sb, kT_sb, 0, m)
            load_w(wv_sb, wv_d, DM)
            nc.sync.dma_start(bvb_sb[:], bvb_d[:])
            nc.gpsimd.memset(v4[:, :, :, HD : HD + 1], 1.0)
            xv = load_xch(xvT_d, 0)
            for mi in range(4):
                proj_v_unit(xv, 0, mi)
            nc.sync.dma_start(cmask_sb[:], cmask_d[:])
            nc.sync.dma_start(ident_sb[:], ident_d[:])
            load_w(wo_sb, wo_d, D)

            for n in range(4):
                # filler PE work to interleave between attention calls:
                # previous chunk's output projection + next chunk's
                # projections
                fillers = []
                if n > 0:
                    fillers.extend(outproj_units(n - 1))
                if n < 3:
                    fillers.extend(proj_units(n + 1))
                fillers = iter(fillers)
                for qt in range(4 * n, 4 * n + 4):
                    o_nat = wrk.tile([128, DM], bf16, tag="o_nat",
                                     name="o_nat", bufs=2)
                    for hp in range(4):
                        attention(hp, qt, o_nat)
                        u = next(fillers, None)
                        if u is not None:
                            u()
                    transpose_o(qt, o_nat)
                for u in fillers:
                    u()
            for u in outproj_units(3):
                u()

            if DEBUG:
                for m in range(4):
                    for nm, t in ((f"dbg_qT{m}", qT_sb[m]),
                                  (f"dbg_kT{m}", kT_sb[m]),
                                  (f"dbg_oT{m}", oT_sb[m])):
                        dd = nc.dram_tensor(nm, [128, S], bf16,
                                            kind="ExternalOutput")
                        nc.sync.dma_start(dd[:], t[:])
                dv = nc.dram_tensor("dbg_v", [128, NQT * HPC * VROW], bf16,
                                    kind="ExternalOutput")
                nc.sync.dma_start(dv[:], v_sb[:])

    nc.compile()
    return nc


def _get_program():
    if "nc" not in _CACHE:
        _CACHE["nc"] = _build_program()
    return _CACHE["nc"]


def _make_in_maps(query, key, value, Wq, bq, Wk, bk, Wv, bv, Wo):
    import ml_dtypes

    bf16 = ml_dtypes.bfloat16
    cmask = np.tile(np.triu(np.ones((128, 128), dtype=np.float32)), (1, 2)).astype(bf16)
    ident = np.eye(128, dtype=np.float32).astype(bf16)
    in_maps = []
    for c in range(NCORES):
        b, g = c // 2, c % 2
        sl = slice(g * DM, (g + 1) * DM)
        in_maps.append({
            "xqT": np.ascontiguousarray(query[b].T),
            "xkT": np.ascontiguousarray(key[b].T),
            "xvT": np.ascontiguousarray(value[b].T),
            "wq": np.ascontiguousarray(Wq[:, sl]),
            "wk": np.ascontiguousarray(Wk[:, sl]),
            "wv": np.ascontiguousarray(Wv[:, sl]),
            "bq": np.ascontiguousarray(bq[sl].reshape(4, 128).T),
            "bk": np.ascontiguousarray(bk[sl].reshape(4, 128).T),
            "bvb": np.ascontiguousarray(
                np.broadcast_to(bv[sl], (128, DM)).astype(np.float32)
            ),
            "wo": np.ascontiguousarray(Wo[sl, :]).astype(bf16),
            "cmask": cmask,
            "ident": ident,
        })
    return in_maps


def _run_spmd(in_maps, trace=False):
    from concourse import bass_utils

    nc = _get_program()
    return bass_utils.run_bass_kernel_spmd(
        nc, in_maps, core_ids=list(range(NCORES)), trace=trace
    )


def _assemble(res, bo):
    out = np.empty((B, S, D), dtype=np.float32)
    bo32 = np.asarray(bo, dtype=np.float32)
    for b in range(B):
        out[b] = (
            res.results[2 * b]["outT"].T
            + res.results[2 * b + 1]["outT"].T
            + bo32
        )
    return out


def _numpy_fallback(query, key, value, mask, Wq, bq, Wk, bk, Wv, bv, Wo, bo):
    """Correct (slow) host path for non-causal masks; never used when the
    mask is the reference's tril."""
    def split_heads(x):
        b, s, _ = x.shape
        return x.reshape(b, s, H, HD).transpose(0, 2, 1, 3)

    q = split_heads(query @ Wq + bq)
    k = split_heads(key @ Wk + bk)
    v = split_heads(value @ Wv + bv)
    nb = query.shape[0]
    out = np.empty((nb, H, S, HD), dtype=np.float32)
    for b in range(nb):
        mb = np.asarray(mask[b, 0]) != 0
        for h in range(H):
            s = (q[b, h] @ k[b, h].T) / math.sqrt(HD)
            s = np.where(mb, s, -np.inf)
            s -= s.max(axis=-1, keepdims=True)
            e = np.exp(s)
            a = e / e.sum(axis=-1, keepdims=True)
            a *= mb
            out[b, h] = a @ v[b, h]
    out = out.transpose(0, 2, 1, 3).reshape(nb, -1, D)
    return (out @ Wo + bo).astype(np.float32)


def kernel(query, key, value, mask, Wq, bq, Wk, bk, Wv, bv, Wo, bo):
    query = np.asarray(query, dtype=np.float32)
    key = np.asarray(key, dtype=np.float32)
    value = np.asarray(value, dtype=np.float32)
    mask = np.asarray(mask)
    Wq = np.asarray(Wq, dtype=np.float32)
    bq = np.asarray(bq, dtype=np.float32)
    Wk = np.asarray(Wk, dtype=np.float32)
    bk = np.asarray(bk, dtype=np.float32)
    Wv = np.asarray(Wv, dtype=np.float32)
    bv = np.asarray(bv, dtype=np.float32)
    Wo = np.asarray(Wo, dtype=np.float32)
    bo = np.asarray(bo, dtype=np.float32)

    causal = np.array_equal(
        np.asarray(mask[0, 0], dtype=np.int32),
        np.tril(np.ones((S, S), dtype=np.int32)),
    ) and all(np.array_equal(mask[b], mask[0]) for b in range(1, mask.shape[0]))
    if not causal:
        return _numpy_fallback(
            query, key, value, mask, Wq, bq, Wk, bk, Wv, bv, Wo, bo
        )

    in_maps = _make_in_maps(query, key, value, Wq, bq, Wk, bk, Wv, bv, Wo)
    res = _run_spmd(in_maps)
    return _assemble(res, bo)


# revision 23
# speedup vs baseline: 3.6062x; 3.6062x over previous
"""Multi-head causal attention (B=4, S=2048, D=1024, H=16) on 8 trn2 NeuronCores.

Sharding: data-parallel over batch (4) x tensor-parallel over heads (2 groups
of 8).  Core c = (b, g) computes, for batch b, head group g:
  QT = (Wq_g^T X_b^T + bq_g)           [512, 2048]  (bf16, head_dim on partitions)
  KT likewise; V = X_b Wv_g + bv_g     [2048, 512]  (bf16, ones column per head)
  per head h, query tile qt (128):
     S^T[k, q] = Kh^T^T Qh^T           (bf16 matmuls, causal block-skipped)
     A^T = exp(S^T / 8)                (ScalarE, k-tiles packed along psum free dim)
     diagonal block masked via bf16 multiply
     psum_o[q, 65] += A^T.T @ V'_h     (col 64 = row sums via ones column)
     O[q, h*64:+64] = psum_o[:, :64] * recip(psum_o[:, 64])
  O^T via PE transpose, out^T = Wo_g^T O^T  (partial, fp32)
Host sums the two per-batch partials and adds bo.
"""

import math

import numpy as np

B, S, D, H = 4, 2048, 1024, 16
HD = D // H          # 64
NCORES = 8
HPC = 8              # heads per core
DM = HPC * HD        # 512 mid-dims per core
NQT = S // 128       # 16 query tiles
KT_PER_EXP = 8       # k-tiles packed into one [128, 1024] psum before exp
VROW = HD + 1        # 65: per-head V columns incl. ones column

_CACHE = {}
DEBUG = False


def _build_program():
    import concourse.mybir as mybir
    import concourse.tile as tile
    from concourse import bacc

    f32 = mybir.dt.float32
    f32r = mybir.dt.float32r
    bf16 = mybir.dt.bfloat16
    EXP = mybir.ActivationFunctionType.Exp

    nc = bacc.Bacc("TRN2", target_bir_lowering=False, debug=False,
                   num_devices=NCORES)

    xqT_d = nc.dram_tensor("xqT", [D, S], bf16, kind="ExternalInput")
    xkT_d = nc.dram_tensor("xkT", [D, S], bf16, kind="ExternalInput")
    xvT_d = nc.dram_tensor("xvT", [D, S], bf16, kind="ExternalInput")
    wq_d = nc.dram_tensor("wq", [D, DM], bf16, kind="ExternalInput")
    wk_d = nc.dram_tensor("wk", [D, DM], bf16, kind="ExternalInput")
    wv_d = nc.dram_tensor("wv", [D, DM], bf16, kind="ExternalInput")
    bq_d = nc.dram_tensor("bq", [128, 4], f32, kind="ExternalInput")
    bk_d = nc.dram_tensor("bk", [128, 4], f32, kind="ExternalInput")
    bvb_d = nc.dram_tensor("bvb", [128, DM], f32, kind="ExternalInput")
    wo_d = nc.dram_tensor("wo", [DM, D], bf16, kind="ExternalInput")
    cmask_d = nc.dram_tensor("cmask", [128, 256], bf16, kind="ExternalInput")
    ident_d = nc.dram_tensor("ident", [128, 128], bf16, kind="ExternalInput")
    outT_d = nc.dram_tensor("outT", [D, S], f32, kind="ExternalOutput")

    with tile.TileContext(nc) as tc:
        with (
            tc.tile_pool(name="res", bufs=1) as res,     # long-lived tensors
            tc.tile_pool(name="wrk", bufs=1) as wrk,     # rotating work tiles
            tc.tile_pool(name="ps", bufs=1, space="PSUM") as ps,
        ):
            # ---- resident tensors -------------------------------------
            wq_sb = res.tile([128, 8 * DM], bf16, tag="wq_sb")
            wk_sb = res.tile([128, 8 * DM], bf16, tag="wk_sb")
            wv_sb = res.tile([128, 8 * DM], bf16, tag="wv_sb")
            wo_sb = res.tile([128, 4 * D], bf16, tag="wo_sb")
            bq_sb = res.tile([128, 4], f32, tag="bq_sb")
            bk_sb = res.tile([128, 4], f32, tag="bk_sb")
            bvb_sb = res.tile([128, DM], f32, tag="bvb_sb")
            cmask_sb = res.tile([128, 256], bf16, tag="cmask_sb")
            ident_sb = res.tile([128, 128], bf16, tag="ident_sb")
            qT_sb = [res.tile([128, S], bf16, tag=f"qT{m}", name=f"qT{m}") for m in range(4)]
            kT_sb = [res.tile([128, S], bf16, tag=f"kT{m}", name=f"kT{m}") for m in range(4)]
            v_sb = res.tile([128, NQT * HPC * VROW], bf16, tag="v_sb")
            oT_sb = [res.tile([128, S], bf16, tag=f"oT{m}", name=f"oT{m}") for m in range(4)]

            def load_w(w_sb, w_d, n_w):
                nc.sync.dma_start(
                    w_sb.rearrange("p (k n) -> p k n", n=n_w),
                    w_d.rearrange("(k p) n -> p k n", p=128),
                )

            v4 = v_sb.rearrange("p (s h c) -> p s h c", h=HPC, c=VROW)
            bvb3 = bvb_sb.rearrange("p (h c) -> p h c", h=HPC)

            # ---- projections, split into per-m-tile units -------------
            def load_xch(xT_d, n):
                xch = wrk.tile([128, 8 * 512], bf16, tag="xch", name="xch",
                               bufs=2)
                nc.sync.dma_start(
                    xch.rearrange("p (k s) -> p k s", k=8),
                    xT_d.rearrange("(k p) s -> p k s", p=128)[
                        :, :, n * 512 : (n + 1) * 512
                    ],
                )
                return xch

            def proj_qk_unit(xch, w_sb, b_sb, dst, n, m):
                pp = ps.tile([128, 512], f32, tag="ps_small", name="pp",
                             bufs=2)
                for kt in range(8):
                    nc.tensor.matmul(
                        pp[:],
                        w_sb[:, kt * DM + m * 128 : kt * DM + (m + 1) * 128],
                        xch[:, kt * 512 : (kt + 1) * 512],
                        start=(kt == 0),
                        stop=(kt == 7),
                    )
                nc.vector.tensor_scalar_add(
                    dst[m][:, n * 512 : (n + 1) * 512], pp[:],
                    b_sb[:, m : m + 1],
                )

            def proj_v_unit(xch, n, mi):
                st = n * 4 + mi          # S tile index
                pp = ps.tile([128, 512], f32, tag="ps_small", name="pp",
                             bufs=2)
                for kt in range(8):
                    nc.tensor.matmul(
                        pp[:],
                        xch[:, kt * 512 + mi * 128 : kt * 512 + (mi + 1) * 128],
                        wv_sb[:, kt * DM : (kt + 1) * DM],
                        start=(kt == 0),
                        stop=(kt == 7),
                    )
                # add bias + scatter heads into 65-wide groups
                nc.vector.tensor_add(
                    v4[:, st, :, 0:HD],
                    pp.rearrange("p (h c) -> p h c", h=HPC),
                    bvb3[:],
                )

            def proj_units(n):
                xq = load_xch(xqT_d, n)
                for m in range(4):
                    yield lambda m=m, x=xq: proj_qk_unit(x, wq_sb, bq_sb,
                                                         qT_sb, n, m)
                xk = load_xch(xkT_d, n)
                for m in range(4):
                    yield lambda m=m, x=xk: proj_qk_unit(x, wk_sb, bk_sb,
                                                         kT_sb, n, m)
                xv = load_xch(xvT_d, n)
                for mi in range(4):
                    yield lambda mi=mi, x=xv: proj_v_unit(x, n, mi)

            # ---- interleaved phases: per S-chunk n: projections,
            # ---- attention on its 4 query tiles, output projection ----
            def attention(hp, qt, o_nat):
                nblk = qt + 1
                heads = (2 * hp, 2 * hp + 1)
                aT = [
                    wrk.tile([128, S], bf16, tag=f"aT{hh}", name=f"aT{hh}", bufs=3)
                    for hh in range(2)
                ]
                # both heads in one 2-bank psum (separate banks: matmul
                # start=True clears the whole bank's has_written flags)
                po = ps.tile([128, 1024], f32, tag="ps_o", name="po", bufs=1)
                pof = [po[:, 0:VROW], po[:, 512 : 512 + VROW]]

                def mm1_exp(blk, cnt):
                    # the two heads sit on PE row-groups 0-63 / 64-127;
                    # alternating their matmuls lets the sub-arrays run
                    # them concurrently (per-subarray LDWEIGHTS overlap)
                    psx = [
                        ps.tile([128, 1024], f32, tag="ps_s",
                                name=f"psx{hh}", bufs=2)
                        for hh in range(2)
                    ]
                    for j in range(cnt):
                        kt = blk * KT_PER_EXP + j
                        for hh in range(2):
                            p0 = hh * 64
                            nc.tensor.matmul(
                                psx[hh][:, j * 128 : (j + 1) * 128],
                                kT_sb[hp][p0 : p0 + 64, kt * 128 : (kt + 1) * 128],
                                qT_sb[hp][p0 : p0 + 64, qt * 128 : (qt + 1) * 128],
                                start=True,
                                stop=True,
                            )
                    for hh in range(2):
                        nc.scalar.activation(
                            aT[hh][:, blk * 1024 : blk * 1024 + cnt * 128],
                            psx[hh][:, : cnt * 128],
                            EXP,
                            scale=1.0 / math.sqrt(HD),
                        )

                def mm2(blk, cnt):
                    for hh in range(2):
                        for j in range(cnt):
                            kt = blk * KT_PER_EXP + j
                            nc.tensor.matmul(
                                pof[hh],
                                aT[hh][:, kt * 128 : (kt + 1) * 128],
                                v4[:, kt, heads[hh], :],
                                start=(kt == 0),
                                stop=(kt == nblk - 1),
                                skip_group_check=True,
                            )

                nexp = (nblk + KT_PER_EXP - 1) // KT_PER_EXP
                cnts = [min(KT_PER_EXP, nblk - b * KT_PER_EXP)
                        for b in range(nexp)]
                for blk in range(nexp):
                    mm1_exp(blk, cnts[blk])
                    if blk == nexp - 1:
                        for hh in range(2):
                            dslc = aT[hh][:, qt * 128 : (qt + 1) * 128]
                            nc.vector.tensor_mul(dslc, dslc,
                                                 cmask_sb[:, 0:128])
                    if blk > 0:
                        mm2(blk - 1, cnts[blk - 1])
                mm2(nexp - 1, cnts[nexp - 1])

                rc = wrk.tile([128, 2], f32, tag="rc", name="rc", bufs=2)
                po_sums = po.rearrange("p (b c) -> p b c", c=512)[:, :, HD : HD + 1]
                nc.vector.reciprocal(rc[:], po_sums)
                for hh in range(2):
                    nc.vector.tensor_scalar_mul(
                        o_nat[:, heads[hh] * HD : (heads[hh] + 1) * HD],
                        pof[hh][:, 0:HD],
                        rc[:, hh : hh + 1],
                    )
                if DEBUG and hp == 0 and qt == 15:
                    for hh in range(2):
                        da = nc.dram_tensor(f"dbg_aT{hh}", [128, S], bf16,
                                            kind="ExternalOutput")
                        nc.sync.dma_start(da[:], aT[hh][:])
                        dpo = nc.dram_tensor(f"dbg_po{hh}", [128, VROW], f32,
                                             kind="ExternalOutput")
                        pos = wrk.tile([128, VROW], f32, tag="dbgpo",
                                       name="dbgpo", bufs=2)
                        nc.vector.tensor_copy(pos[:], pof[hh])
                        nc.sync.dma_start(dpo[:], pos[:])

            def transpose_o(qt, o_nat):
                for m in range(4):
                    pt = ps.tile([128, 512], bf16, tag="ps_small", bufs=2)
                    nc.tensor.transpose(
                        pt[:, :128],
                        o_nat[:, m * 128 : (m + 1) * 128],
                        ident_sb[:],
                    )
                    nc.vector.tensor_copy(
                        oT_sb[m][:, qt * 128 : (qt + 1) * 128], pt[:, :128]
                    )

            def outproj_unit(n, m8):
                pp = ps.tile([128, 512], f32, tag="ps_small", name="pp",
                             bufs=2)
                for kt in range(4):
                    nc.tensor.matmul(
                        pp[:],
                        wo_sb[:, kt * D + m8 * 128 : kt * D + (m8 + 1) * 128],
                        oT_sb[kt][:, n * 512 : (n + 1) * 512],
                        start=(kt == 0),
                        stop=(kt == 3),
                    )
                ost = wrk.tile([128, 512], f32, tag="ost", name="ost", bufs=2)
                nc.vector.tensor_copy(ost[:], pp[:])
                nc.sync.dma_start(
                    outT_d[m8 * 128 : (m8 + 1) * 128, n * 512 : (n + 1) * 512],
                    ost[:],
                )

            def outproj_units(n):
                for m8 in range(8):
                    yield lambda m8=m8: outproj_unit(n, m8)

            # batch 0: emit projections for n=0 up front, DMAs ordered
            # so the first matmul's operands arrive first
            units0 = []
            load_w(wq_sb, wq_d, DM)
            nc.sync.dma_start(bq_sb[:], bq_d[:])
            xq = load_xch(xqT_d, 0)
            for m in range(4):
                proj_qk_unit(xq, wq_sb, bq_sb, qT_sb, 0, m)
            load_w(wk_sb, wk_d, DM)
            nc.sync.dma_start(bk_sb[:], bk_d[:])
            xk = load_xch(xkT_d, 0)
            for m in range(4):
                proj_qk_unit(xk, wk_sb, bk_sb, kT_sb, 0, m)
            load_w(wv_sb, wv_d, DM)
            nc.sync.dma_start(bvb_sb[:], bvb_d[:])
            nc.gpsimd.memset(v4[:, :, :, HD : HD + 1], 1.0)
            xv = load_xch(xvT_d, 0)
            for mi in range(4):
                proj_v_unit(xv, 0, mi)
            nc.sync.dma_start(cmask_sb[:], cmask_d[:])
            nc.sync.dma_start(ident_sb[:], ident_d[:])
            load_w(wo_sb, wo_d, D)

            for n in range(4):
                # filler PE work to interleave between attention calls:
                # previous chunk's output projection + next chunk's
                # projections
                fillers = []
                if n > 0:
                    fillers.extend(outproj_units(n - 1))
                if n < 3:
                    fillers.extend(proj_units(n + 1))
                fillers = iter(fillers)
                for qt in range(4 * n, 4 * n + 4):
                    o_nat = wrk.tile([128, DM], bf16, tag="o_nat",
                                     name="o_nat", bufs=2)
                    for hp in range(4):
                        attention(hp, qt, o_nat)
                        u = next(fillers, None)
                        if u is not None:
                            u()
                    transpose_o(qt, o_nat)
                for u in fillers:
                    u()
            for u in outproj_units(3):
                u()

            if DEBUG:
                for m in range(4):
                    for nm, t in ((f"dbg_qT{m}", qT_sb[m]),
                                  (f"dbg_kT{m}", kT_sb[m]),
                                  (f"dbg_oT{m}", oT_sb[m])):
                        dd = nc.dram_tensor(nm, [128, S], bf16,
                                            kind="ExternalOutput")
                        nc.sync.dma_start(dd[:], t[:])
                dv = nc.dram_tensor("dbg_v", [128, NQT * HPC * VROW], bf16,
                                    kind="ExternalOutput")
                nc.sync.dma_start(dv[:], v_sb[:])

    nc.compile()
    return nc


def _get_program():
    if "nc" not in _CACHE:
        _CACHE["nc"] = _build_program()
    return _CACHE["nc"]


def _make_in_maps(query, key, value, Wq, bq, Wk, bk, Wv, bv, Wo):
    import ml_dtypes

    bf16 = ml_dtypes.bfloat16
    cmask = np.tile(np.triu(np.ones((128, 128), dtype=np.float32)), (1, 2)).astype(bf16)
    ident = np.eye(128, dtype=np.float32).astype(bf16)
    in_maps = []
    for c in range(NCORES):
        b, g = c // 2, c % 2
        sl = slice(g * DM, (g + 1) * DM)
        in_maps.append({
            "xqT": np.ascontiguousarray(query[b].T).astype(bf16),
            "xkT": np.ascontiguousarray(key[b].T).astype(bf16),
            "xvT": np.ascontiguousarray(value[b].T).astype(bf16),
            "wq": np.ascontiguousarray(Wq[:, sl]).astype(bf16),
            "wk": np.ascontiguousarray(Wk[:, sl]).astype(bf16),
            "wv": np.ascontiguousarray(Wv[:, sl]).astype(bf16),
            "bq": np.ascontiguousarray(bq[sl].reshape(4, 128).T),
            "bk": np.ascontiguousarray(bk[sl].reshape(4, 128).T),
            "bvb": np.ascontiguousarray(
                np.broadcast_to(bv[sl], (128, DM)).astype(np.float32)
            ),
            "wo": np.ascontiguousarray(Wo[sl, :]).astype(bf16),
            "cmask": cmask,
            "ident": ident,
        })
    return in_maps


def _run_spmd(in_maps, trace=False):
    from concourse import bass_utils

    nc = _get_program()
    return bass_utils.run_bass_kernel_spmd(
        nc, in_maps, core_ids=list(range(NCORES)), trace=trace
    )


def _assemble(res, bo):
    out = np.empty((B, S, D), dtype=np.float32)
    bo32 = np.asarray(bo, dtype=np.float32)
    for b in range(B):
        out[b] = (
            res.results[2 * b]["outT"].T
            + res.results[2 * b + 1]["outT"].T
            + bo32
        )
    return out


def _numpy_fallback(query, key, value, mask, Wq, bq, Wk, bk, Wv, bv, Wo, bo):
    """Correct (slow) host path for non-causal masks; never used when the
    mask is the reference's tril."""
    def split_heads(x):
        b, s, _ = x.shape
        return x.reshape(b, s, H, HD).transpose(0, 2, 1, 3)

    q = split_heads(query @ Wq + bq)
    k = split_heads(key @ Wk + bk)
    v = split_heads(value @ Wv + bv)
    nb = query.shape[0]
    out = np.empty((nb, H, S, HD), dtype=np.float32)
    for b in range(nb):
        mb = np.asarray(mask[b, 0]) != 0
        for h in range(H):
            s = (q[b, h] @ k[b, h].T) / math.sqrt(HD)
            s = np.where(mb, s, -np.inf)
            s -= s.max(axis=-1, keepdims=True)
            e = np.exp(s)
            a = e / e.sum(axis=-1, keepdims=True)
            a *= mb
            out[b, h] = a @ v[b, h]
    out = out.transpose(0, 2, 1, 3).reshape(nb, -1, D)
    return (out @ Wo + bo).astype(np.float32)


def kernel(query, key, value, mask, Wq, bq, Wk, bk, Wv, bv, Wo, bo):
    query = np.asarray(query, dtype=np.float32)
    key = np.asarray(key, dtype=np.float32)
    value = np.asarray(value, dtype=np.float32)
    mask = np.asarray(mask)
    Wq = np.asarray(Wq, dtype=np.float32)
    bq = np.asarray(bq, dtype=np.float32)
    Wk = np.asarray(Wk, dtype=np.float32)
    bk = np.asarray(bk, dtype=np.float32)
    Wv = np.asarray(Wv, dtype=np.float32)
    bv = np.asarray(bv, dtype=np.float32)
    Wo = np.asarray(Wo, dtype=np.float32)
    bo = np.asarray(bo, dtype=np.float32)

    causal = np.array_equal(
        np.asarray(mask[0, 0], dtype=np.int32),
        np.tril(np.ones((S, S), dtype=np.int32)),
    ) and all(np.array_equal(mask[b], mask[0]) for b in range(1, mask.shape[0]))
    if not causal:
        return _numpy_fallback(
            query, key, value, mask, Wq, bq, Wk, bk, Wv, bv, Wo, bo
        )

    in_maps = _make_in_maps(query, key, value, Wq, bq, Wk, bk, Wv, bv, Wo)
    res = _run_spmd(in_maps)
    return _assemble(res, bo)
